# revision 12
# baseline (speedup 1.0000x reference)
"""AttentionGRU Trainium2 kernel — 8-core data-parallel over batch.

Reference math (per batch b):
  fWr = facts @ Wr.T; fW = facts @ W.T            (precompute GEMMs)
  per step t:
    r   = sigmoid(fWr_t + h @ Ur_w.T + Ur_b)
    h_t = tanh(fW_t + r * (h @ U_w.T + U_b))
    h   = g_t * h_t + (1 - g_t) * h
  output = states[num_facts-1]

Kernel strategy:
  - Shard batch B=128 over 8 cores (16 sequences/core); weights replicated.
  - Host-side: zero g[b, t] for t >= num_facts[b]  => final h IS the answer
    (no gather needed on device).
  - Transposed layout everywhere on device: feature dim on the 128
    partitions (8 tiles of 128), batch on the free dim (16).
    h tile: (128, 8*16) where free = jd*16 + b.
  - Matmuls: weights stationary (lhsT = W.T tile, 128x128 bf16, FWL),
    rhs = h tiles (128,16) streaming; f32 PSUM accumulation over 8 d-tiles.
  - Ur_b folded into fWr at precompute; U_b pre-broadcast on host.
"""

import os
import numpy as np
import ml_dtypes

import concourse.bass as bass
import concourse.mybir as mybir
import concourse.tile as tile
from concourse import bacc
from concourse.bass_utils import run_bass_kernel_spmd

B, T, D = 128, 128, 1024
NCORES = 8
BL = B // NCORES          # 16 local batch
JD = D // 128             # 8 feature tiles
NT = T * BL               # 2048 free size of (t, b)

F32 = mybir.dt.float32
BF16 = mybir.dt.bfloat16
bfnp = ml_dtypes.bfloat16

_cache = {}
last_exec_time_ns = None


def build_nc():
    nc = bacc.Bacc()

    factsT_d = nc.declare_dram_parameter("factsT", [JD, 128, NT], BF16, isOutput=False)
    wrT_d = nc.declare_dram_parameter("wrT", [JD, 128, D], BF16, isOutput=False)
    wT_d = nc.declare_dram_parameter("wT", [JD, 128, D], BF16, isOutput=False)
    ucatT_d = nc.declare_dram_parameter("ucatT", [JD, 128, 2 * D], BF16, isOutput=False)
    urbb_d = nc.declare_dram_parameter("urbb", [128, JD * BL], F32, isOutput=False)
    ubb_d = nc.declare_dram_parameter("ubb", [128, JD * BL], F32, isOutput=False)
    g_d = nc.declare_dram_parameter("g", [128, T, BL], F32, isOutput=False)
    h0_d = nc.declare_dram_parameter("h0", [128, JD * BL], F32, isOutput=False)
    out_d = nc.declare_dram_parameter("out", [128, JD * BL], F32, isOutput=True)

    SIG = mybir.ActivationFunctionType.Sigmoid
    TANH = mybir.ActivationFunctionType.Tanh

    with tile.TileContext(nc) as tc:
        with (
            tc.tile_pool(name="consts", bufs=1) as consts,
            tc.tile_pool(name="phase1", bufs=1) as phase1,
            tc.tile_pool(name="acts", bufs=1) as acts,
            tc.tile_pool(name="hpool", bufs=3) as hpool,
            tc.tile_pool(name="hbf", bufs=3) as hbfpool,
            tc.tile_pool(name="tmp", bufs=2) as tmp,
            tc.tile_pool(name="pre_ps", bufs=4, space="PSUM") as pre_ps,
            tc.tile_pool(name="r_ps", bufs=2, space="PSUM") as r_ps,
            tc.tile_pool(name="u_ps", bufs=2, space="PSUM") as u_ps,
        ):
            # ---- constant / input tiles ----
            ucatT = consts.tile([128, JD, 2 * D], BF16)
            urbb = consts.tile([128, JD * BL], F32)
            ubb = consts.tile([128, JD * BL], F32)
            g_sb = consts.tile([128, T, BL], F32)
            factsT = phase1.tile([128, JD, NT], BF16)
            wrT = phase1.tile([128, JD, D], BF16)
            wT = phase1.tile([128, JD, D], BF16)
            fWrT = acts.tile([128, T, JD * BL], BF16)
            fWT = acts.tile([128, T, JD * BL], BF16)

            nc.sync.dma_start(out=factsT[:], in_=factsT_d[:].transpose([1, 0, 2]))
            nc.sync.dma_start(out=wrT[:], in_=wrT_d[:].transpose([1, 0, 2]))
            nc.sync.dma_start(out=wT[:], in_=wT_d[:].transpose([1, 0, 2]))
            nc.sync.dma_start(out=ucatT[:], in_=ucatT_d[:].transpose([1, 0, 2]))
            nc.sync.dma_start(out=urbb[:], in_=urbb_d[:])
            nc.sync.dma_start(out=ubb[:], in_=ubb_d[:])
            nc.sync.dma_start(out=g_sb[:], in_=g_d[:])
            h_cur = hpool.tile([128, JD * BL], F32, tag="h")
            nc.sync.dma_start(out=h_cur[:], in_=h0_d[:])

            # ---- precompute fWrT (+Ur_b) and fWT ----
            NCH = 4  # chunks of 512 over (t,b)
            CH = NT // NCH  # 512
            TC = CH // BL  # 32 t per chunk
            for w_idx, (wsb, dest) in enumerate(((wrT, fWrT), (wT, fWT))):
                for jm in range(JD):
                    for c in range(NCH):
                        ps = pre_ps.tile([128, CH], F32, tag="pre")
                        for jd in range(JD):
                            nc.tensor.matmul(
                                ps[:],
                                lhsT=wsb[:, jd, jm * 128 : (jm + 1) * 128],
                                rhs=factsT[:, jd, c * CH : (c + 1) * CH],
                                start=(jd == 0),
                                stop=(jd == JD - 1),
                            )
                        dest_sl = dest[:, c * TC : (c + 1) * TC, jm * BL : (jm + 1) * BL]
                        ps_v = ps[:].rearrange("p (t b) -> p t b", b=BL)
                        nc.vector.tensor_copy(dest_sl, ps_v)

            # ---- recurrence ----
            for t in range(T):
                hbf = hbfpool.tile([128, JD * BL], BF16, tag="hbf")
                nc.vector.tensor_copy(hbf[:], h_cur[:])

                pr = r_ps.tile([128, JD * BL], F32, tag="pr")
                pu = u_ps.tile([128, JD * BL], F32, tag="pu")
                for jm in range(JD):
                    for jd in range(JD):
                        nc.tensor.matmul(
                            pr[:, jm * BL : (jm + 1) * BL],
                            lhsT=ucatT[:, jd, jm * 128 : (jm + 1) * 128],
                            rhs=hbf[:, jd * BL : (jd + 1) * BL],
                            start=(jd == 0),
                            stop=(jd == JD - 1),
                        )
                for jm in range(JD):
                    for jd in range(JD):
                        nc.tensor.matmul(
                            pu[:, jm * BL : (jm + 1) * BL],
                            lhsT=ucatT[:, jd, D + jm * 128 : D + (jm + 1) * 128],
                            rhs=hbf[:, jd * BL : (jd + 1) * BL],
                            start=(jd == 0),
                            stop=(jd == JD - 1),
                        )

                # epilogue, full (128, 128) granularity
                tr = tmp.tile([128, JD * BL], F32, tag="tr")
                nc.vector.tensor_add(tr[:], pr[:], fWrT[:, t, :])
                tr2 = tmp.tile([128, JD * BL], F32, tag="tr2")
                nc.vector.tensor_add(tr2[:], tr[:], urbb[:])
                r = tmp.tile([128, JD * BL], F32, tag="r")
                nc.scalar.activation(r[:], tr2[:], SIG)
                up = tmp.tile([128, JD * BL], F32, tag="up")
                nc.vector.tensor_add(up[:], pu[:], ubb[:])
                ru = tmp.tile([128, JD * BL], F32, tag="ru")
                nc.vector.tensor_mul(ru[:], r[:], up[:])
                v = tmp.tile([128, JD * BL], F32, tag="v")
                nc.vector.tensor_add(v[:], ru[:], fWT[:, t, :])
                ht = tmp.tile([128, JD * BL], F32, tag="ht")
                nc.scalar.activation(ht[:], v[:], TANH)
                dl = tmp.tile([128, JD * BL], F32, tag="dl")
                nc.vector.tensor_sub(dl[:], ht[:], h_cur[:])
                gd = tmp.tile([128, JD * BL], F32, tag="gd")
                g_t = g_sb[:, t : t + 1, :].broadcast_to([128, JD, BL])
                nc.vector.tensor_mul(
                    gd[:].rearrange("p (j b) -> p j b", b=BL),
                    dl[:].rearrange("p (j b) -> p j b", b=BL),
                    g_t,
                )
                h_new = hpool.tile([128, JD * BL], F32, tag="h")
                nc.vector.tensor_add(h_new[:], h_cur[:], gd[:])
                h_cur = h_new

            nc.sync.dma_start(out=out_d[:], in_=h_cur[:])

    nc.finalize()
    return nc


def _prep(inputs):
    facts = np.ascontiguousarray(np.asarray(inputs["facts"], dtype=np.float32))
    num_facts = np.asarray(inputs["num_facts"]).astype(np.int64)
    g = np.asarray(inputs["g"], dtype=np.float32)
    mem_old = np.asarray(inputs["mem_old"], dtype=np.float32)
    Wr = np.asarray(inputs["Wr"], dtype=np.float32)
    Ur_w = np.asarray(inputs["Ur_w"], dtype=np.float32)
    Ur_b = np.asarray(inputs["Ur_b"], dtype=np.float32)
    W = np.asarray(inputs["W"], dtype=np.float32)
    U_w = np.asarray(inputs["U_w"], dtype=np.float32)
    U_b = np.asarray(inputs["U_b"], dtype=np.float32)

    # shared (replicated) arrays
    wrT = np.ascontiguousarray(Wr.T).reshape(JD, 128, D).astype(bfnp)
    wT = np.ascontiguousarray(W.T).reshape(JD, 128, D).astype(bfnp)
    ucatT = np.ascontiguousarray(
        np.concatenate([Ur_w.T, U_w.T], axis=1)
    ).reshape(JD, 128, 2 * D).astype(bfnp)
    urbb = np.ascontiguousarray(
        np.repeat(Ur_b.reshape(JD, 128).T[:, :, None], BL, axis=2).reshape(128, JD * BL)
    ).astype(np.float32)
    ubb = np.ascontiguousarray(
        np.repeat(U_b.reshape(JD, 128).T[:, :, None], BL, axis=2).reshape(128, JD * BL)
    ).astype(np.float32)

    # g zeroed past num_facts (makes final h == states[num_facts-1]);
    # num_facts<1 or >T behave like the reference's gather (wrap/clamp to T-1).
    nf_eff = np.where(num_facts < 1, T, np.minimum(num_facts, T))
    g2 = g[:, :, 0].copy()
    g2[np.arange(T)[None, :] >= nf_eff[:, None]] = 0.0

    in_maps = []
    for c in range(NCORES):
        s = slice(c * BL, (c + 1) * BL)
        factsT = np.ascontiguousarray(
            facts[s].transpose(2, 1, 0)
        ).reshape(JD, 128, NT).astype(bfnp)
        g_b = np.ascontiguousarray(
            np.broadcast_to(g2[s].T[None, :, :], (128, T, BL))
        ).astype(np.float32)
        h0 = np.ascontiguousarray(
            mem_old[s, 0, :].T.reshape(JD, 128, BL).transpose(1, 0, 2)
        ).reshape(128, JD * BL).astype(np.float32)
        in_maps.append(
            {
                "factsT": factsT,
                "wrT": wrT,
                "wT": wT,
                "ucatT": ucatT,
                "urbb": urbb,
                "ubb": ubb,
                "g": g_b,
                "h0": h0,
            }
        )
    return in_maps


def kernel(**inputs) -> np.ndarray:
    global last_exec_time_ns
    if "nc" not in _cache:
        _cache["nc"] = build_nc()
    nc = _cache["nc"]
    in_maps = _prep(inputs)
    trace = bool(int(os.environ.get("BASS_KERNEL_TRACE", "0")))
    kw = {}
    if trace:
        kw["trace"] = True
        kw["tmpdir"] = os.environ.get("BASS_KERNEL_TMPDIR") or None
    res = run_bass_kernel_spmd(nc, in_maps, core_ids=list(range(NCORES)), **kw)
    last_exec_time_ns = res.exec_time_ns
    outs = []
    for c in range(NCORES):
        o = np.asarray(res.results[c]["out"], dtype=np.float32)  # (128, JD*BL)
        o = o.reshape(128, JD, BL).transpose(1, 0, 2).reshape(D, BL).T  # (BL, D)
        outs.append(o)
    return np.ascontiguousarray(np.concatenate(outs, axis=0))
